# revision 25
# baseline (speedup 1.0000x reference)
"""Weighted 2D Gaussian KDE on 8 Trainium2 NeuronCores (Bass/Tile), v2.

out[b,l] = sum_n w[n] * exp(-||x[b,l] - data[n]||^2 / sigma),  sigma = 3.

Grid-quadrature factorization (see kernel v1 docstring): with a uniform
grid u_j (spacing h, a = 2/sigma, F = h*sqrt(2a/pi)),
    out[c] = F^2 * q0(x_c)^T (P1 diag(w) P0^T) q1(x_c),
    P_d[j,n] = exp(-a(u_j - d_nd)^2),  q_d[j,c] = exp(-a(u_j - x_cd)^2).

v2 redesign (informed by the CoreSim cost model):
  * All matmuls in bf16 (1 cyc/col vs 4 for fp32). Accuracy is kept by
    hi/lo bf16 splitting on the host: every large product u*x or x^2 is
    expressed as a sum of exact-in-fp32 bf16-pair products; the arg error
    is ~1e-4, far below the exp-scale that matters.
  * Feature rows (x_hi, x_lo, a*x^2 hi/lo, ln w, ...) are precomputed on
    the host as bf16 row tensors: elementwise O(n) prep that removes the
    25us/row partition-collapse DMAs of v1.
  * Stage A computes arg in [points, grid] orientation (features as the
    stationary operand), so the moment-matrix accumulation needs no DMA
    transposes at all.
  * Exps batched [128, 1024] across 2 PSUM banks to amortize activation
    init overhead. The w and F^2 factors fold into the ln-w feature row.
  * Output rows accumulate in a single [32, 512] PSUM tile (one matmul
    per strip, partition-offset write) DMA'd straight to DRAM once.
  * R-mul (q0 * (M q1)) split DVE/GpSimd to balance engine load.

Sharding: locations (B*L = 131072) split contiguously across 8 cores
(16384 each); data/weights replicated; moment matrix computed
redundantly on every core (collectives cost >=15us fixed - not worth it
for a 16KB AllReduce).
"""

import os
import numpy as np
import ml_dtypes

import concourse.bass as bass
import concourse.tile as tile
from concourse import bacc
from concourse import mybir
from concourse import bass_utils

BF = ml_dtypes.bfloat16

# ---- problem constants (hardcoded per spec) ----
B, L, D = 2, 65536, 2
NPTS = 16384
NCORES = 8
NLOC = B * L              # 131072 locations
NSH = NLOC // NCORES      # 16384 per core
SIGMA = 3.0
A = 2.0 / SIGMA
PADG = 3.0                # grid extension beyond data/location range
NG = 64                   # partition stride per dim (dim0 rows 0:NGR,
                          # dim1 rows NG:NG+NGR — bases must be 0/64)
NGR = 48                  # real grid node count; spacing adapts to range
HMIN = 0.75               # spacing floor (denser helps nothing here)
HMAX = 1.02               # quadrature error 2exp(-pi^2/(2A h^2)) ~ 1.6e-3

KX = 12                   # featx rows (padded to 16 in the wide layout)
KD = 16                   # featd rows
WIDE = 2048               # wide feature layout: 2 blocks of 64 partitions,
                          # each holding 4 groups of 16 feature rows
NSTRIP = 512
NSTRIPS = NSH // NSTRIP        # 32 location strips
NCHUNK = NPTS // 128           # 128 data chunks
AGRP = 16                      # stage-A chunks per exp batch
BGRP = 2                       # stage-B strips per exp batch
W0 = 20                        # stage-A dim0 grid window (nodes) per chunk
RPAD = 4.0                     # window reach beyond a chunk's d0 range

F32 = mybir.dt.float32
BF16 = mybir.dt.bfloat16
AF = mybir.ActivationFunctionType


def _build_core_program(nc: bass.Bass, w0, o0s):
    featx = nc.dram_tensor("featx", [128, WIDE], BF16, kind="ExternalInput").ap()
    featd = nc.dram_tensor("featd", [128, WIDE], BF16, kind="ExternalInput").ap()
    ga = nc.dram_tensor("ga", [128, 8 * NG], BF16, kind="ExternalInput").ap()
    gb = nc.dram_tensor("gb", [128, 8 * NG], BF16, kind="ExternalInput").ap()
    ubias = nc.dram_tensor("ubias", [2 * NG, 1], F32, kind="ExternalInput").ap()
    out = nc.dram_tensor("out", [NSH], F32, kind="ExternalOutput").ap()

    with tile.TileContext(nc) as tc:
        with (
            tc.tile_pool(name="const", bufs=1) as const,
            tc.tile_pool(name="sbA", bufs=2) as sbA,
            tc.tile_pool(name="sbQ", bufs=6) as sbQ,
            tc.tile_pool(name="sbR", bufs=3) as sbR,
        ):
            # featd halves + ga go first on separate DGE queues so stage A
            # can start as early as possible (DMA fixed latency ~2.8us).
            # Nothing issues on the Activation queue: its sequencer must be
            # free for the first exp.
            featd_sb = const.tile([128, WIDE], BF16)
            Q4 = WIDE // 4
            nc.sync.dma_start(out=featd_sb[:, 0:Q4], in_=featd[:, 0:Q4])
            ga_sb = const.tile([128, 8 * NG], BF16)
            nc.sync.dma_start(out=ga_sb, in_=ga)
            for qi, qeng in [(1, nc.gpsimd), (2, nc.sync), (3, nc.gpsimd)]:
                qeng.dma_start(
                    out=featd_sb[:, qi * Q4 : (qi + 1) * Q4],
                    in_=featd[:, qi * Q4 : (qi + 1) * Q4],
                )
            gb_sb = const.tile([128, 8 * NG], BF16)
            nc.sync.dma_start(out=gb_sb, in_=gb)
            ubias_sb = const.tile([2 * NG, 1], F32)
            nc.sync.dma_start(out=ubias_sb, in_=ubias)
            featx_sb = const.tile([128, WIDE], BF16)
            nc.sync.dma_start(out=featx_sb, in_=featx)
            ones_sb = const.tile([NGR, 1], BF16)
            nc.vector.memset(ones_sb, 1.0)
            # Warm the Exp table while input DMAs run.
            warm = const.tile([1, 1], F32)
            nc.vector.memset(warm, 0.0)
            warm2 = const.tile([1, 1], F32)
            nc.scalar.activation(warm2, warm, AF.Exp)

            # -------- stage A: moment matrix M2w[k,j] = sum_n P1 P0 --------
            # argT chunks [128 pts, 128 grid] via lhsT = featd slice.
            # m2 lives on partitions 64:128 so the stage-B T-matmul's lhsT
            # base matches its rhs (qb bottom half) — matmul requires equal
            # lhsT/rhs base partitions.
            pa_ctx = tc.tile_pool(name="psA", bufs=2, space="PSUM")
            psA = pa_ctx.__enter__()
            pm_ctx = tc.tile_pool(name="psM", bufs=1, space="PSUM")
            psM = pm_ctx.__enter__()
            m2ps = psM.tile([128, NG], F32, tag="m2", bufs=1)
            # Chunk emission order: ascending j-quartile, so the first
            # groups only need the first quarter of featd (cols j*128) and
            # stage A starts as soon as that quarter-DMA lands.
            chorder = sorted(range(NCHUNK), key=lambda c: ((c % 16) // 4, c))
            gsizes = [AGRP] * (NCHUNK // AGRP)
            gstart = [sum(gsizes[:i]) for i in range(len(gsizes))]

            # Group tile layout: per-chunk dim1 args (NGR cols) then the
            # w0-col dim0 windows, packed left-to-right but never letting a
            # single matmul write cross a 512-col PSUM bank boundary.
            def colpack(n):
                offs, gaps, cur = [], [], 0
                for w in [NGR] * n + [w0] * n:
                    if cur % 512 + w > 512:
                        nxt = (cur // 512 + 1) * 512
                        gaps.append((cur, nxt - cur))
                        cur = nxt
                    offs.append(cur)
                    cur += w
                return offs, gaps, cur

            zg = const.tile([1, 512], BF16)
            nc.vector.memset(zg, 0.0)

            def emit_argT(g):
                n = gsizes[g]
                offs, gaps, ACOLS = colpack(n)
                at = psA.tile([128, ACOLS], F32, tag="argT")
                for goff, gw in gaps:
                    # bank-alignment gap: zero-fill so the batched exp never
                    # reads uninitialized PSUM
                    nc.tensor.matmul(
                        at[:, goff : goff + gw], lhsT=zg[:, 0:128],
                        rhs=zg[:, 0:gw], start=True, stop=True,
                    )
                for i in range(n):
                    ch = chorder[gstart[g] + i]
                    blk, grp, j = ch // 64, (ch // 16) % 4, ch % 16
                    bs = slice(blk * 64, (blk + 1) * 64)
                    gcol = grp * 128
                    nc.tensor.matmul(
                        at[:, offs[i] : offs[i] + NGR],
                        lhsT=featd_sb[bs, j * 128 : (j + 1) * 128],
                        rhs=ga_sb[bs, gcol : gcol + NGR],
                        start=True, stop=True,
                    )
                    o0 = o0s[ch]
                    nc.tensor.matmul(
                        at[:, offs[n + i] : offs[n + i] + w0],
                        lhsT=featd_sb[bs, j * 128 : (j + 1) * 128],
                        rhs=ga_sb[bs, gcol + 64 + o0 : gcol + 64 + o0 + w0],
                        start=True, stop=True,
                    )
                return at, offs, n

            # zero the m2 accumulator once; windowed accums then read-modify
            zz = const.tile([1, NG], BF16)
            nc.vector.memset(zz, 0.0)
            nc.tensor.matmul(
                m2ps[NG : 2 * NG, :], lhsT=zz, rhs=zz, start=True, stop=False,
                skip_group_check=True,
            )
            NGA = len(gsizes)
            ats = {0: emit_argT(0)}
            for g in range(NGA):
                at, offs, n = ats.pop(g)
                pat = sbA.tile([128, at.shape[1]], BF16, tag="PAT")
                nc.scalar.activation(pat, at, AF.Exp)
                if g + 1 < NGA:
                    ats[g + 1] = emit_argT(g + 1)
                for i in range(n):
                    ch = chorder[gstart[g] + i]
                    nc.tensor.matmul(
                        m2ps[NG : NG + NGR, o0s[ch] : o0s[ch] + w0],
                        lhsT=pat[:, offs[i] : offs[i] + NGR],
                        rhs=pat[:, offs[n + i] : offs[n + i] + w0],
                        start=False,
                        stop=(gstart[g] + i == NCHUNK - 1),
                        skip_group_check=True,
                    )
            m2bf = const.tile([128, NG], BF16)
            nc.scalar.mul(m2bf[NG : NG + NGR, :], m2ps[NG : NG + NGR, :], 1.0)
            pm_ctx.__exit__(None, None, None)
            pa_ctx.__exit__(None, None, None)

            # -------- stage B: per-location evaluation --------
            pb_ctx = tc.tile_pool(name="psB", bufs=2, space="PSUM")
            psB = pb_ctx.__enter__()
            pt_ctx = tc.tile_pool(name="psT", bufs=3, space="PSUM")
            psT = pt_ctx.__enter__()
            po_ctx = tc.tile_pool(name="psO", bufs=1, space="PSUM")
            psO = po_ctx.__enter__()
            # Final reduce: out_col[p] = sum_j rb[j, i*128+p] via matmul with
            # ones as the MOVING operand (N=1, ~free on PE). Columns collect
            # in one [128, 128] PSUM tile; the host permutes locations so
            # this tile DMAs to DRAM contiguously.
            ocol = psO.tile([128, NSTRIPS * 4], F32, tag="oc", bufs=1)
            NGB = NSTRIPS // BGRP

            def emit_argB(g):
                ap2 = psB.tile([128, BGRP * NSTRIP], F32, tag="argB")
                for i in range(BGRP):
                    s = g * BGRP + i
                    blk, grp, j = s // 16, (s // 4) % 4, s % 4
                    bs = slice(blk * 64, (blk + 1) * 64)
                    nc.tensor.matmul(
                        ap2[:, i * NSTRIP : (i + 1) * NSTRIP],
                        lhsT=gb_sb[bs, grp * 128 : (grp + 1) * 128],
                        rhs=featx_sb[bs, j * NSTRIP : (j + 1) * NSTRIP],
                        start=True, stop=True,
                    )
                return ap2

            aps = {0: emit_argB(0), 1: emit_argB(1)}
            out2 = out.rearrange("(p q) -> p q", p=128)
            for g in range(NGB):
                qb = sbQ.tile([128, BGRP * NSTRIP], BF16, tag="QB")
                nc.scalar.activation(qb, aps.pop(g), AF.Exp, bias=ubias_sb)
                if g + 2 < NGB:
                    aps[g + 2] = emit_argB(g + 2)
                for i in range(BGRP):
                    s = g * BGRP + i
                    sl = slice(i * NSTRIP, (i + 1) * NSTRIP)
                    tps = psT.tile([NGR, NSTRIP], F32, tag="T")
                    nc.tensor.matmul(
                        tps,
                        lhsT=m2bf[NG : NG + NGR, 0:NGR],
                        rhs=qb[NG : NG + NGR, sl],
                        start=True, stop=True,
                    )
                    rb = sbR.tile([NGR, NSTRIP], BF16, tag="R")
                    if s % 8 == 5:
                        # GPSIMD can't read PSUM (walrus rule): stage T
                        # through SBUF via the scalar engine, mul on Pool.
                        tcp = sbR.tile([NGR, NSTRIP], BF16, tag="TC")
                        nc.scalar.activation(tcp, tps, AF.Copy)
                        nc.gpsimd.tensor_mul(rb, qb[0:NGR, sl], tcp)
                    else:
                        nc.vector.tensor_mul(rb, qb[0:NGR, sl], tps)
                    for i in range(4):
                        nc.tensor.matmul(
                            ocol[:, 4 * s + i : 4 * s + i + 1],
                            lhsT=rb[:, i * 128 : (i + 1) * 128],
                            rhs=ones_sb,
                            start=True, stop=True, skip_group_check=True,
                        )
                if g % 4 == 3:
                    # a quarter of the output is complete: drain it early so
                    # only the last quarter's copy+DMA sits in the tail
                    qtr = g // 4
                    cs = slice(qtr * NSTRIPS, (qtr + 1) * NSTRIPS)
                    osb1 = const.tile([128, NSTRIPS], F32, tag=f"osb{qtr}")
                    nc.vector.tensor_copy(osb1, ocol[:, cs])
                    nc.sync.dma_start(out=out2[:, cs], in_=osb1)
            po_ctx.__exit__(None, None, None)
            pt_ctx.__exit__(None, None, None)
            pb_ctx.__exit__(None, None, None)
    return nc


_CACHE = {}
LAST_RESULTS = None
LAST_O0S = None


def _get_nc():
    key = LAST_O0S
    if key not in _CACHE:
        nc = bacc.Bacc("TRN2", target_bir_lowering=False, debug=False)
        _build_core_program(nc, key[0], key[1])
        nc.compile()
        _CACHE[key] = nc
    return _CACHE[key]


def _split(v):
    """bf16 hi/lo split of float64 v: hi + lo == v to ~16 mantissa bits."""
    hi = v.astype(BF)
    lo = (v - hi.astype(np.float64)).astype(BF)
    return hi, lo


def _host_prep(x, data, weights):
    """Elementwise O(n) host prep: bf16 feature rows + grid coefficient
    matrices. All transcendental/reduction work stays on-device."""
    lo = float(min(x.min(), data.min())) - PADG
    hi = float(max(x.max(), data.max())) + PADG
    H = max(HMIN, (hi - lo) / (NGR - 1))
    assert H <= 1.1, f"range {hi - lo} too wide for {NGR} nodes"
    FQ = float(H * np.sqrt(2.0 * A / np.pi))
    u = lo + np.arange(NGR) * H

    c_hi, c_lo = _split(2.0 * A * u)          # [NGR] each
    b_hi, b_lo = _split(-A * u * u)
    # layout rows per dim: 0:NGR real nodes, NGR:NG inert pad (zero
    # coefficients + -4900 bias -> exp underflows to exactly 0)
    ub1 = np.full(NG, -4900.0, dtype=np.float32)
    ub1[:NGR] = (-A * u * u).astype(np.float32)
    ubias = np.tile(ub1, 2)[:, None]

    # gb rows [KX, 2*NG]: per-dim 6-row blocks pairing with featx rows
    #   rows (dim d block at 6d..6d+5): x_hi*c_hi, x_hi*c_lo, x_lo*c_hi,
    #   x_lo*c_lo, s_hi*(-1), s_lo*(-1); cols 0:NG dim0, NG:2NG dim1.
    gb = np.zeros((KX, 2 * NG), dtype=np.float64)
    for d in range(2):
        cs = slice(d * NG, d * NG + NGR)
        gb[6 * d + 0, cs] = c_hi.astype(np.float64)
        gb[6 * d + 1, cs] = c_lo.astype(np.float64)
        gb[6 * d + 2, cs] = c_hi.astype(np.float64)
        gb[6 * d + 3, cs] = c_lo.astype(np.float64)
        gb[6 * d + 4, cs] = -1.0
        gb[6 * d + 5, cs] = -1.0
    # ga [KD, 2*NG]: col block 0:NG = dim1 grid, NG:2NG = dim0 grid (the
    # windowed one); rows 0..11 pair data features, 12,13 ln-w coeff (on
    # the dim0 block), 14,15 grid bias via ones features.
    ga = np.zeros((KD, 2 * NG), dtype=np.float64)
    ga[:6, NG : NG + NGR] = gb[:6, :NGR]      # dim0 features -> dim0 cols
    ga[6:12, :NGR] = gb[6:12, NG : NG + NGR]  # dim1 features -> dim1 cols
    ga[12, NG : NG + NGR] = 1.0
    ga[13, NG : NG + NGR] = 1.0
    bh = np.zeros(NG)
    bh[:NGR] = b_hi.astype(np.float64)
    bl = np.zeros(NG)
    bl[:NGR] = b_lo.astype(np.float64)
    ga[14, :] = np.tile(bh, 2)
    ga[15, :] = np.tile(bl, 2)

    def feat6(v):
        v = v.astype(np.float64)
        v_hi, v_lo = _split(v)
        s = A * v * v
        s_hi, s_lo = _split(s)
        return [v_hi, v_hi, v_lo, v_lo, s_hi, s_lo]

    x64 = x.reshape(NLOC, D).astype(np.float64)
    featx = np.stack(feat6(x64[:, 0]) + feat6(x64[:, 1]))  # [12, NLOC] bf16

    # Sort data by d0 so each 128-point chunk has a narrow d0 range; its
    # dim0 grid support then fits a W0-node window (reach +-4.0 =>
    # truncation ~5e-5). Falls back to the full 64-node window (same
    # code, offsets 0) if the data distribution ever defeats this.
    d64 = data.astype(np.float64)
    dperm = np.argsort(d64[:, 0], kind="stable")
    dsort = d64[dperm]
    wsort = weights.astype(np.float64)[dperm]
    d0c = dsort[:, 0].reshape(NCHUNK, 128)
    lo_node = np.floor((d0c.min(axis=1) - RPAD - lo) / H).astype(int)
    hi_node = np.ceil((d0c.max(axis=1) + RPAD - lo) / H).astype(int)
    if (hi_node - lo_node + 1).max() <= W0:
        w0 = W0
        o0s = np.clip(lo_node, 0, NGR - W0)
    else:
        w0 = NGR
        o0s = np.zeros(NCHUNK, dtype=int)

    lnw = np.log(np.maximum(wsort, 1e-300))
    lnw = np.maximum(lnw + 2.0 * np.log(FQ), -60.0)
    lnw_hi, lnw_lo = _split(lnw)
    ones = np.ones(NPTS, dtype=BF)
    featd = np.stack(
        feat6(dsort[:, 0]) + feat6(dsort[:, 1]) + [lnw_hi, lnw_lo, ones, ones]
    )  # [16, NPTS] bf16

    return featx.astype(BF), featd.astype(BF), ga, gb, ubias, w0, o0s


def _widen(rows, n):
    """[K<=16, n] -> [128, n//8]: fully packed, 8 column-groups of n//8.
    Partition g*16+k holds feature row k of column-group g; groups 0-3 sit
    in the base-0 block of 64 partitions, groups 4-7 in the base-64 block
    (the only legal matmul base partitions)."""
    k = rows.shape[0]
    w = np.zeros((8, 16, n // 8), dtype=BF)
    w[:, :k] = rows.reshape(k, 8, n // 8).transpose(1, 0, 2)
    return w.reshape(128, n // 8)


def _grep(g):
    """Zero-banded replicas of a [K<=16, m] coefficient matrix: variant
    `grp` (columns grp*m..) is nonzero only on the 16-row band that group
    occupies inside its 64-partition block, so a K=64 contraction picks
    out exactly one packed feature group."""
    m = g.shape[1]
    r = np.zeros((8, 16, 4, m), dtype=np.float64)
    for grp in range(4):
        for blk in range(2):
            r[blk * 4 + grp, : g.shape[0], grp] = g
    return r.reshape(128, 4 * m).astype(BF)


# Location permutation within each core shard: featx column
# (s*512 + i*128 + p) holds shard location (p*128 + 4s + i), so the final
# [128 lanes, 128 reduce-columns] output tile maps to DRAM row-major.
def _locperm():
    cols = np.arange(NSH)
    s, r = cols // NSTRIP, cols % NSTRIP
    i, p = r // 128, r % 128
    return p * 128 + 4 * s + i


LOCPERM = _locperm()


def make_in_maps(x, data, weights):
    featx, featd, ga, gb, ubias, w0, o0s = _host_prep(x, data, weights)
    global LAST_O0S
    LAST_O0S = (int(w0), tuple(int(v) for v in o0s))
    featd_w = _widen(featd, NPTS)
    ga_w, gb_w = _grep(ga), _grep(gb)
    in_maps = []
    for c in range(NCORES):
        in_maps.append({
            "featx": _widen(featx[:, c * NSH + LOCPERM], NSH),
            "featd": featd_w,
            "ga": ga_w,
            "gb": gb_w,
            "ubias": ubias,
        })
    return in_maps


def kernel(x, data, weights):
    global LAST_RESULTS
    x = np.ascontiguousarray(x, dtype=np.float32)
    data = np.ascontiguousarray(data, dtype=np.float32)
    weights = np.ascontiguousarray(weights, dtype=np.float32)
    assert x.shape == (B, L, D) and data.shape == (NPTS, D)

    in_maps = make_in_maps(x, data, weights)
    nc = _get_nc()  # program depends on LAST_O0S set by make_in_maps
    try:
        res = bass_utils.run_bass_kernel_spmd(
            nc, in_maps, core_ids=list(range(NCORES)),
            trace=bool(os.environ.get("BASS_TRACE")),
        )
    except ModuleNotFoundError:
        # BASS_TRACE set but this container lacks the NTFF profile hook
        # (antenv.axon_hooks) — rerun without tracing.
        res = bass_utils.run_bass_kernel_spmd(
            nc, in_maps, core_ids=list(range(NCORES)), trace=False,
        )
    LAST_RESULTS = res
    out = np.concatenate([res.results[c]["out"] for c in range(NCORES)])
    return out.reshape(B, L)
